# revision 4
# baseline (speedup 1.0000x reference)
"""DVAE GNN message-passing kernel for 8 Trainium2 NeuronCores.

Data parallel over batch B=2048 -> 256 graphs/core; 20-step topological scan.

v3 design: fully feature-major ("flipped") layout.
- Every activation lives transposed in SBUF fp16 tiles [128, 1024] with
  cols = chunk*256 + half*128 + b (4 feature chunks of 128/128/128/117,
  two 128-sample halves). Matmuls run stationary=WEIGHT chunks, moving=
  activation chunks: out = W^T-chunk^T @ actT-chunk lands feature-major in
  PSUM at 128 cols (one half) per matmul. No PE transposes exist at all.
- The two sample halves run as offset pipelines: while half A does its
  sigmoid/tanh/blend chain on Scalar/DVE, the PE runs half B's matmuls,
  so the PE never idles and holds its fast p-state.
- Biases fold in for free: b_ih+b_hh ride an extra ones-row of X^T through
  the x-part matmul; b_hh_n rides row 117 of the WHH chunk-3 stationary
  against an always-1.0 row 117 of the accumulator; bg + the per-step
  vertex-id contribution ride a tiny 21-row stationary (WGVA/WMVA) against
  a broadcast one-hot column.
- Message scatter: Hin accumulators ACCT[v] are feature-major fp16 SBUF,
  initialized host-side with the constant part sum_{u>=v} adj*C_u. Each
  pair (u,vp) adds GT_u * adj[:,u,vp] via two fp16 2x-mode tensor_tensor
  ops (mult into tmp, add into ACCT) on DVE or GpSimd; the adj row is a
  host-replicated [128, 256] tile streamed from HBM in per-step groups,
  broadcast across chunks via a middle-dim stride-0 AP view.
- Stationary weight slices are always 128 cols wide (zero-padded), so
  every matmul writes all 128 PSUM partitions and chunk-3 junk rows are
  exact zeros: G's junk rows come out 0 and the scatter preserves the
  accumulator's 1.0 bias row.
"""

import sys
import numpy as np

for _p in ("/opt/trn_rl_repo",):
    if _p not in sys.path:
        sys.path.insert(0, _p)

B, MAXN, NVT, HS, NZ = 2048, 20, 26, 501, 56
NCORES = 8
BS = B // NCORES              # 256 samples per core
NVT_EFF = NVT + MAXN          # 46
XD = NVT_EFF + 1              # 47 (one-hot + feat scalar)
G3 = 3 * HS                   # 1503
CW = [128, 128, 128, 117]     # feature chunk widths (sum 501)
KW = [128, 128, 128, 118]     # gh k widths (chunk 3 includes bias row 117)
CO = [0, 128, 256, 384]       # chunk feature offsets

POOL_TARGETS = frozenset({10, 13, 16, 19})
DVE_CAP = 7
POOL_CAP = 3


def _scatter_schedule():
    """EDF schedule of deferred scatter pairs.

    Pair (u, vp), vp >= u+2: available step u+1, must run during some step
    s <= vp-1 (before the critical add of (vp-1, vp) emitted at the end of
    step vp-1). Targets in POOL_TARGETS run on GpSimd, the rest on DVE.
    Returns sched[s] = list of (u, vp, route) for s in 0..MAXN-1 (route
    0=DVE, 1=pool).
    """
    import heapq

    hd, hp = [], []
    sched = [[] for _ in range(MAXN)]
    for s in range(1, MAXN - 1):
        u = s - 1
        for vp in range(u + 2, MAXN):
            heapq.heappush(hp if vp in POOL_TARGETS else hd, (vp, u))
        for heap, cap, route in ((hd, DVE_CAP, 0), (hp, POOL_CAP, 1)):
            n = 0
            while heap and (n < cap or heap[0][0] <= s + 1):
                vp, uu = heapq.heappop(heap)
                assert vp >= s + 1, f"missed deadline: ({uu},{vp}) at step {s}"
                sched[s].append((uu, vp, route))
                n += 1
    assert not hd and not hp
    return sched


def _adj_groups():
    """DMA groups of adj tiles: groups[s] = [(s, s+1) crit] + sched[s] pairs.

    Returns (groups, index) where index[(u, vp)] = (s, i) giving the group
    and tile slot of each pair's [128, 256] adj tile.
    """
    sched = _scatter_schedule()
    groups, index = [], {}
    for s in range(MAXN - 1):
        g = [(s, s + 1)] + [(u, vp) for (u, vp, _r) in sched[s]]
        for i, pair in enumerate(g):
            index[pair] = (s, i)
        groups.append(g)
    return groups, index


def _wpack_layout():
    """Column layout (fp16) of the packed static tensor. All entries use
    128 rows (zero-padded). Returns (entries, ncols): name -> (col0, ncols).
    """
    ents = {}
    col = 0

    def put(name, ncols):
        nonlocal col
        ents[name] = (col, ncols)
        col += ncols

    for c in range(4):
        put(f"whh{c}", 1536)
    put("wih", 1536)
    for c in range(4):
        put(f"wg{c}", 512)
    for c in range(4):
        put(f"wm{c}", 512)
    put("wgva", 512)
    put("wmva", 512)
    put("vidc", MAXN)
    for c in range(4):
        put(f"w12{c}", 128)
    put("xt", MAXN * BS)
    for v in range(MAXN):
        put(f"acct{v}", 1024)
    return ents, col


_PROG = None


def _build_program():
    import concourse.bass as bass  # noqa: F401
    import concourse.tile as tile
    from concourse import bacc, mybir

    f16 = mybir.dt.float16
    f32 = mybir.dt.float32
    AF = mybir.ActivationFunctionType
    OP = mybir.AluOpType

    nc = bacc.Bacc("TRN2", target_bir_lowering=False, debug=False)

    sched = _scatter_schedule()
    groups, gindex = _adj_groups()
    goff = []  # column offset of each group in d_adjs
    off = 0
    for g in groups:
        goff.append(off)
        off += 256 * len(g)
    adjs_cols = off
    maxg = max(len(g) for g in groups)

    ents, wcols = _wpack_layout()
    d_wpack = nc.dram_tensor("wpack", [128, wcols], f16, kind="ExternalInput").ap()
    d_adjs = nc.dram_tensor("adjs", [128, adjs_cols], f16, kind="ExternalInput").ap()
    d_bout = nc.dram_tensor("bout", [128, 1], f32, kind="ExternalInput").ap()
    d_out = nc.dram_tensor("out", [112, BS], f32, kind="ExternalOutput").ap()

    def mm(out, lhsT, rhs, start, stop):
        nc.tensor.matmul(out, lhsT, rhs, start=start, stop=stop)

    with tile.TileContext(nc) as tc:
        with (
            tc.tile_pool(name="statics", bufs=1) as sp,
            tc.tile_pool(name="accs", bufs=1) as accp,
            tc.tile_pool(name="gstore", bufs=1) as gp,
            tc.tile_pool(name="adjg", bufs=3) as adjp,
            tc.tile_pool(name="work", bufs=2) as wp,
            tc.tile_pool(name="tmps", bufs=4) as tp,
            tc.tile_pool(name="ps", bufs=8, space="PSUM") as pp,
        ):
            def sl(name):
                c0, ncl = ents[name]
                return d_wpack[:, c0:c0 + ncl]

            WHH = [sp.tile([128, 1536], f16, tag=f"whh{c}", name=f"whh{c}")
                   for c in range(4)]
            WIH = sp.tile([128, 1536], f16, tag="wih", name="wih")
            WG = [sp.tile([128, 512], f16, tag=f"wg{c}", name=f"wg{c}")
                  for c in range(4)]
            WM = [sp.tile([128, 512], f16, tag=f"wm{c}", name=f"wm{c}")
                  for c in range(4)]
            WGVA = sp.tile([128, 512], f16, tag="wgva", name="wgva")
            WMVA = sp.tile([128, 512], f16, tag="wmva", name="wmva")
            VIDC = sp.tile([128, MAXN], f16, tag="vidc", name="vidc")
            W12 = [sp.tile([128, 128], f16, tag=f"w12{c}", name=f"w12{c}")
                   for c in range(4)]
            XT = sp.tile([128, MAXN * BS], f16, tag="xt", name="xt")
            BOUT = sp.tile([128, 1], f32, tag="bout", name="bout")

            # weights needed from step 0 first, then xt+accts
            for c in range(4):
                nc.sync.dma_start(WHH[c][:, :], sl(f"whh{c}"))
            nc.sync.dma_start(WIH[:, :], sl("wih"))
            nc.sync.dma_start(XT[:, :], sl("xt"))
            for c in range(4):
                nc.sync.dma_start(WG[c][:, :], sl(f"wg{c}"))
                nc.sync.dma_start(WM[c][:, :], sl(f"wm{c}"))
            nc.sync.dma_start(WGVA[:, :], sl("wgva"))
            nc.sync.dma_start(WMVA[:, :], sl("wmva"))
            nc.sync.dma_start(VIDC[:, :], sl("vidc"))
            for c in range(4):
                nc.sync.dma_start(W12[c][:, :], sl(f"w12{c}"))
            nc.sync.dma_start(BOUT[:, :], d_bout[:, :])

            ACCT = [accp.tile([128, 1024], f16, tag=f"acct{v}", name=f"acct{v}")
                    for v in range(MAXN)]
            for v in range(MAXN):
                nc.sync.dma_start(ACCT[v][:, :], sl(f"acct{v}"))

            GT = [gp.tile([128, 1024], f16, tag=f"gt{u}", name=f"gt{u}")
                  for u in range(MAXN - 1)]

            ADJG = {}

            def load_group(s):
                if s >= len(groups) or not groups[s]:
                    return
                t = adjp.tile([128, maxg * 256], f16, tag="adjg", name=f"adjg{s}")
                n = len(groups[s])
                nc.sync.dma_start(t[:, :256 * n], d_adjs[:, goff[s]:goff[s] + 256 * n])
                ADJG[s] = t

            load_group(0)
            load_group(1)

            def hv(t, h):
                """strided half view [128, 4, 128] of a [128, 1024] tile"""
                r = t.rearrange("p (c hb) -> p c hb", c=4)
                return r[:, :, h * 128:(h + 1) * 128]

            def fv(t):
                """full view [128, 4, 256]"""
                return t.rearrange("p (c hb) -> p c hb", c=4)

            def adj_ap(pair, h=None):
                """adj operand view for a pair: middle-dim-0 broadcast"""
                s, i = gindex[pair]
                t = ADJG[s]
                if h is None:
                    a = t[:, i * 256:(i + 1) * 256]
                    return a.unsqueeze(1).broadcast_to([128, 4, 256])
                a = t[:, i * 256 + h * 128:i * 256 + h * 128 + 128]
                return a.unsqueeze(1).broadcast_to([128, 4, 128])

            def scatter_pair(u, vp, route):
                """deferred full-width MAC: ACCT[vp] += GT[u] * adj"""
                eng = nc.vector if route == 0 else nc.gpsimd
                t = tp.tile([128, 1024], f16, tag=f"sc{route}", name=f"sc{u}_{vp}")
                eng.tensor_tensor(fv(t), fv(GT[u]), adj_ap((u, vp)), OP.mult)
                eng.tensor_tensor(fv(ACCT[vp]), fv(ACCT[vp]), fv(t), OP.add)

            # ---- x-part of the n gate for step v, half h -> inT tile ----
            def emit_xin(v, h, INT):
                ps = pp.tile([128, 512], f32, tag="ps", name=f"in{v}_{h}")
                for m in range(4):
                    mm(ps[:, m * 128:(m + 1) * 128],
                       WIH[0:XD + 1, 2 * 512 + m * 128:2 * 512 + (m + 1) * 128],
                       XT[0:XD + 1, v * BS + h * 128:v * BS + (h + 1) * 128],
                       start=True, stop=True)
                nc.scalar.copy(hv(INT, h), ps.rearrange("p (m b) -> p m b", m=4))

            # prologue: inT for step 0
            INT = wp.tile([128, 1024], f16, tag="inT", name="inT0")
            for h in range(2):
                emit_xin(0, h, INT)

            HT_final = [None, None]
            OUTPS = [None]

            for v in range(MAXN):
                rT = wp.tile([128, 1024], f16, tag="rT", name=f"rT{v}")
                zT = wp.tile([128, 1024], f16, tag="zT", name=f"zT{v}")
                hnT = wp.tile([128, 1024], f16, tag="hnT", name=f"hnT{v}")
                qT = wp.tile([128, 1024], f16, tag="qT", name=f"qT{v}")
                nT = wp.tile([128, 1024], f16, tag="nT", name=f"nT{v}")
                dT = wp.tile([128, 1024], f16, tag="dT", name=f"dT{v}")
                hT = wp.tile([128, 1024], f16, tag="hT", name=f"hT{v}")
                sgT = wp.tile([128, 1024], f16, tag="sgT", name=f"sgT{v}")
                mpT = wp.tile([128, 1024], f16, tag="mpT", name=f"mpT{v}")
                curINT = INT
                if v + 1 < MAXN:
                    INT = wp.tile([128, 1024], f16, tag="inT", name=f"inT{v + 1}")

                djobs = sched[v]
                dsplit = [djobs[i::3] for i in range(3)]

                def emit_gh(h):
                    # r and z gates: x-mm (bias row) + 4 h-chunk mms per m
                    for g in range(2):
                        ps = pp.tile([128, 512], f32, tag="ps",
                                     name=f"g{g}_{v}_{h}")
                        for m in range(4):
                            dst = ps[:, m * 128:(m + 1) * 128]
                            wc = g * 512 + m * 128
                            mm(dst, WIH[0:XD + 1, wc:wc + 128],
                               XT[0:XD + 1, v * BS + h * 128:v * BS + (h + 1) * 128],
                               start=True, stop=False)
                            for c in range(4):
                                mm(dst, WHH[c][0:KW[c], wc:wc + 128],
                                   ACCT[v][0:KW[c], c * 256 + h * 128:
                                           c * 256 + (h + 1) * 128],
                                   start=False, stop=(c == 3))
                        dst_t = rT if g == 0 else zT
                        nc.scalar.activation(hv(dst_t, h),
                                             ps.rearrange("p (m b) -> p m b", m=4),
                                             AF.Sigmoid)
                    # hn: h-part of n gate (bias b_hh_n via WHH[3] row 117)
                    ps = pp.tile([128, 512], f32, tag="ps", name=f"hn{v}_{h}")
                    for m in range(4):
                        dst = ps[:, m * 128:(m + 1) * 128]
                        wc = 2 * 512 + m * 128
                        for c in range(4):
                            mm(dst, WHH[c][0:KW[c], wc:wc + 128],
                               ACCT[v][0:KW[c], c * 256 + h * 128:
                                       c * 256 + (h + 1) * 128],
                               start=(c == 0), stop=(c == 3))
                    nc.scalar.copy(hv(hnT, h), ps.rearrange("p (m b) -> p m b", m=4))

                def emit_elem(h):
                    # n = tanh(inT + rT*hnT); h = n + z*(hin - n)
                    nc.vector.tensor_tensor(hv(qT, h), hv(rT, h), hv(hnT, h),
                                            OP.mult)
                    nc.vector.tensor_tensor(hv(qT, h), hv(qT, h),
                                            hv(curINT, h), OP.add)
                    nc.scalar.activation(hv(nT, h), hv(qT, h), AF.Tanh)
                    nc.vector.tensor_tensor(hv(dT, h), hv(ACCT[v], h),
                                            hv(nT, h), OP.subtract)
                    nc.vector.tensor_tensor(hv(dT, h), hv(zT, h), hv(dT, h),
                                            OP.mult)
                    nc.vector.tensor_tensor(hv(hT, h), hv(dT, h), hv(nT, h),
                                            OP.add)

                def emit_zpmp(h):
                    # gated message for vertex v: G = sigmoid(zp)*mp
                    vsel = VIDC[0:21, v:v + 1].broadcast_to([21, 128])
                    for which, WX, WXV, dst_t in ((0, WG, WGVA, sgT),
                                                  (1, WM, WMVA, mpT)):
                        ps = pp.tile([128, 512], f32, tag="ps",
                                     name=f"zp{which}_{v}_{h}")
                        for m in range(4):
                            dst = ps[:, m * 128:(m + 1) * 128]
                            mm(dst, WXV[0:21, m * 128:(m + 1) * 128], vsel,
                               start=True, stop=False)
                            for c in range(4):
                                mm(dst, WX[c][0:CW[c], m * 128:(m + 1) * 128],
                                   hT[0:CW[c], c * 256 + h * 128:
                                      c * 256 + (h + 1) * 128],
                                   start=False, stop=(c == 3))
                        ps3 = ps.rearrange("p (m b) -> p m b", m=4)
                        if which == 0:
                            nc.scalar.activation(hv(dst_t, h), ps3, AF.Sigmoid)
                        else:
                            nc.scalar.copy(hv(dst_t, h), ps3)
                    nc.vector.tensor_tensor(hv(GT[v], h), hv(sgT, h),
                                            hv(mpT, h), OP.mult)

                def emit_crit(h):
                    t = tp.tile([128, 1024], f16, tag="crit", name=f"cr{v}_{h}")
                    nc.vector.tensor_tensor(hv(t, h), hv(GT[v], h),
                                            adj_ap((v, v + 1), h), OP.mult)
                    nc.vector.tensor_tensor(hv(ACCT[v + 1], h),
                                            hv(ACCT[v + 1], h), hv(t, h),
                                            OP.add)

                def emit_readout(h):
                    if h == 0:
                        OUTPS[0] = pp.tile([128, 512], f32, tag="ps",
                                           name="outps")
                    ps = OUTPS[0]
                    for c in range(4):
                        mm(ps[:, h * 128:(h + 1) * 128],
                           W12[c][0:CW[c], 0:128],
                           hT[0:CW[c], c * 256 + h * 128:c * 256 + (h + 1) * 128],
                           start=(c == 0), stop=(c == 3))

                # ---------------- step emission (halves interleaved) -------
                emit_gh(0)
                for (u, vp, r) in dsplit[0]:
                    scatter_pair(u, vp, r)
                emit_elem(0)
                emit_gh(1)
                for (u, vp, r) in dsplit[1]:
                    scatter_pair(u, vp, r)
                emit_elem(1)
                if v + 1 < MAXN:
                    emit_xin(v + 1, 0, INT)
                if v < MAXN - 1:
                    emit_zpmp(0)
                    emit_crit(0)
                else:
                    emit_readout(0)
                for (u, vp, r) in dsplit[2]:
                    scatter_pair(u, vp, r)
                if v + 1 < MAXN:
                    emit_xin(v + 1, 1, INT)
                if v < MAXN - 1:
                    emit_zpmp(1)
                    emit_crit(1)
                else:
                    emit_readout(1)
                load_group(v + 2)

            # ---- readout epilogue: bias + store ----
            ob = wp.tile([128, 256], f32, tag="ob", name="ob")
            nc.scalar.activation(ob[:, :], OUTPS[0][:, :256], AF.Identity,
                                 bias=BOUT[:, 0:1])
            nc.sync.dma_start(d_out[:, :], ob[0:112, :])

    nc.compile()
    return nc


def _host_prep(types, feats, adj, Wg, bg, Wm, W_ih, b_ih, W_hh, b_hh, W1, b1, W2, b2):
    """Build per-core input maps (numpy only)."""
    f = np.float32
    f16 = np.float16
    types = np.asarray(types).astype(np.int64)
    feats = np.asarray(feats, dtype=f)
    adj = np.asarray(adj, dtype=f)
    Wg, bg, Wm = np.asarray(Wg, f), np.asarray(bg, f), np.asarray(Wm, f)
    W_ih, b_ih = np.asarray(W_ih, f), np.asarray(b_ih, f)
    W_hh, b_hh = np.asarray(W_hh, f), np.asarray(b_hh, f)
    W1, b1 = np.asarray(W1, f), np.asarray(b1, f)
    W2, b2 = np.asarray(W2, f), np.asarray(b2, f)

    bsz = types.shape[0]
    bs = bsz // NCORES

    ents, wcols = _wpack_layout()
    groups, _gindex = _adj_groups()

    # ---- gate-column mapping helpers ----
    def gcolmap(W, nrows):
        """W [G3, k] -> [nrows, 1536] with cols g*512 + m*128 + j."""
        out = np.zeros((nrows, 1536), dtype=f)
        WT = W.T  # [k, G3]
        for g in range(3):
            for m in range(4):
                w = CW[m]
                out[:, g * 512 + m * 128:g * 512 + m * 128 + w] = \
                    WT[:, g * HS + CO[m]:g * HS + CO[m] + w]
        return out

    whh_map = gcolmap(W_hh, HS)           # [501, 1536] rows = hidden features
    wih_full = np.zeros((128, 1536), dtype=f)
    wih_full[:XD] = gcolmap(W_ih, XD)     # rows = x dims

    # bias folding: row XD (=47) of wih carries b_ih (+ b_hh for r/z)
    brz = b_ih + b_hh
    for g in range(3):
        src = brz if g < 2 else b_ih
        for m in range(4):
            w = CW[m]
            wih_full[XD, g * 512 + m * 128:g * 512 + m * 128 + w] = \
                src[g * HS + CO[m]:g * HS + CO[m] + w]

    # whh chunk tiles; chunk 3 row 117 = b_hh_n on n-gate cols
    whh_tiles = []
    for c in range(4):
        t = np.zeros((128, 1536), dtype=f)
        t[:CW[c]] = whh_map[CO[c]:CO[c] + CW[c]]
        if c == 3:
            for m in range(4):
                w = CW[m]
                t[117, 2 * 512 + m * 128:2 * 512 + m * 128 + w] = \
                    b_hh[2 * HS + CO[m]:2 * HS + CO[m] + w]
        whh_tiles.append(t)

    def hcolmap(W):
        """W [HS, HS] (out x in): -> per-k-chunk tiles [128, 512] cols m*128+j"""
        WT = W.T  # [in, out]
        tiles = []
        for c in range(4):
            t = np.zeros((128, 512), dtype=f)
            for m in range(4):
                w = CW[m]
                t[:CW[c], m * 128:m * 128 + w] = \
                    WT[CO[c]:CO[c] + CW[c], CO[m]:CO[m] + w]
            tiles.append(t)
        return tiles

    wg_tiles = hcolmap(Wg[:, :HS])
    wm_tiles = hcolmap(Wm[:, :HS])

    def vamap(W, brow):
        """vid part [HS, MAXN] -> [128, 512]: rows 0..19 vid, row 20 bias"""
        t = np.zeros((128, 512), dtype=f)
        WT = W.T  # [MAXN, HS]
        for m in range(4):
            w = CW[m]
            t[:MAXN, m * 128:m * 128 + w] = WT[:, CO[m]:CO[m] + w]
            t[MAXN, m * 128:m * 128 + w] = brow[CO[m]:CO[m] + w]
        return t

    wgva = vamap(Wg[:, HS:], bg)
    wmva = vamap(Wm[:, HS:], np.zeros(HS, f))

    vidc = np.zeros((128, MAXN), dtype=f)
    vidc[:MAXN] = np.eye(MAXN, dtype=f)
    vidc[MAXN] = 1.0

    w12_tiles = []
    for c in range(4):
        t = np.zeros((128, 128), dtype=f)
        t[:CW[c], :NZ] = W1.T[CO[c]:CO[c] + CW[c]]
        t[:CW[c], NZ:2 * NZ] = W2.T[CO[c]:CO[c] + CW[c]]
        w12_tiles.append(t)

    bout = np.zeros((128, 1), dtype=f)
    bout[:NZ, 0] = b1
    bout[NZ:2 * NZ, 0] = b2

    # constant gated vectors for zero hidden state
    zg = 1.0 / (1.0 + np.exp(-(bg[None, :] + Wg[:, HS:].T)))   # [20, 501]
    C = (zg * Wm[:, HS:].T).astype(f)                           # [20, 501]

    # X^T with ones row
    X = np.zeros((bsz, MAXN, XD + 1), dtype=f)
    onehot = np.eye(NVT_EFF, dtype=f)[types.reshape(-1) % NVT_EFF]
    X[:, :, :NVT_EFF] = onehot.reshape(bsz, MAXN, NVT_EFF)
    X[:, :, NVT_EFF] = feats
    X[:, :, XD] = 1.0

    umask = (np.arange(MAXN)[:, None] >= np.arange(MAXN)[None, :]).astype(f)

    def pack_actT(a):
        """[HS, 256] -> [128, 1024] cols c*256 + h*128 + b"""
        out = np.zeros((128, 1024), dtype=f)
        for c in range(4):
            w = CW[c]
            out[:w, c * 256:c * 256 + 256] = a[CO[c]:CO[c] + w, :]
        return out

    in_maps = []
    for core in range(NCORES):
        slc = slice(core * bs, (core + 1) * bs)
        adjc = adj[slc]                      # [256, 20, 20]
        Xc = X[slc]                          # [256, 20, 48]

        wpack = np.zeros((128, wcols), dtype=f16)

        def place(name, arr):
            c0, ncl = ents[name]
            assert arr.shape == (128, ncl), (name, arr.shape, ncl)
            wpack[:, c0:c0 + ncl] = arr.astype(f16)

        for c in range(4):
            place(f"whh{c}", whh_tiles[c])
            place(f"wg{c}", wg_tiles[c])
            place(f"wm{c}", wm_tiles[c])
            place(f"w12{c}", w12_tiles[c])
        place("wih", wih_full)
        place("wgva", wgva)
        place("wmva", wmva)
        place("vidc", vidc)

        xt = np.zeros((128, MAXN * bs), dtype=f)
        # cols v*256 + h*128 + b ; contiguous b over 256 == v*256 + b256
        xt[:XD + 1] = Xc.transpose(2, 1, 0).reshape(XD + 1, MAXN * bs)
        place("xt", xt)

        # acct const init: sum_{u>=v} adj[b,u,v] * C[u]  (feature-major)
        for v in range(MAXN):
            adjm = adjc[:, :, v] * umask[None, :, v]       # [256, 20]
            acc = np.einsum("bu,uf->fb", adjm, C)          # [501, 256]
            t = pack_actT(acc)
            t[117, 768:1024] = 1.0                         # bias row for gh
            place(f"acct{v}", t)

        # adj scatter tiles, grouped by schedule step
        total = sum(len(g) for g in groups)
        adjs = np.zeros((128, total * 256), dtype=f16)
        colp = 0
        for g in groups:
            for (u, vp) in g:
                row = adjc[:, u, vp].astype(f16)           # [256]
                adjs[:, colp:colp + 256] = row[None, :]
                colp += 256
        in_maps.append(dict(wpack=wpack, adjs=adjs, bout=bout))
    return in_maps


def _get_prog():
    global _PROG
    if _PROG is None:
        _PROG = _build_program()
    return _PROG


def kernel(**inputs):
    from concourse.bass_utils import run_bass_kernel_spmd
    nc = _get_prog()
    in_maps = _host_prep(**inputs)
    res = run_bass_kernel_spmd(nc, in_maps, core_ids=list(range(NCORES)))
    outs = []
    for r in res.results:
        o = r["out"]                      # [112, 256] feature-major
        outs.append(np.ascontiguousarray(o.T))
    out = np.concatenate(outs, axis=0)    # [2048, 112]
    mu = np.ascontiguousarray(out[:, :NZ]).astype(np.float32)
    logvar = np.ascontiguousarray(out[:, NZ:2 * NZ]).astype(np.float32)
    return mu, logvar
